# revision 15
# baseline (speedup 1.0000x reference)
"""3-layer LSTM (B=256, T=512, I=128, H=64) + final linear, on 8 TRN2 NeuronCores.

G=2 pipelined variant: each core's 32-batch is split into two independent
16-batch chains (A, B) offset by OFFSET ns.  Each chain runs the skew-2
3-layer wavefront with its own gate tile [128, 96], cell state [64, 48],
PSUM bank pair, and semaphores; the two chains share the engines, weights
and x-chunk DMAs.  While chain A sits in a cross-engine latency gap
(write-commit tails + semaphore props dominate the serial recurrence),
chain B's work executes on the idle engines.

Per-engine instruction emission follows the nominal steady-state schedule
(period PERIOD, chain B at +OFFSET) because engine queues are FIFO with
head-of-line blocking: emission order must equal execution order.

Sync design (per chain): every instruction carries at most ONE attached
sem wait; WAR/WAW covered transitively by engine order; same-engine RAW
(X,Q -> C) gets an explicit self-wait because DVE writes commit only
after the pipeline drain.
"""
import numpy as np
import ml_dtypes

B, T, I, H = 256, 512, 128, 64
NCORES = 8
BC = B // NCORES            # 32 batch per core
G = 2
BG = BC // G                # 16 batch per chain
NB = 3 * BG                 # 48 cols per chain
XCHUNK = 16
NXBUF = 3

PERIOD = 1856.0
OFFSET = 480.0

BF16 = np.float16
_cache = {}

_permA = np.r_[64:128, 0:64]       # [f; i]
_permB = np.r_[192:256, 128:192]   # [o; g]
_sA = np.full(128, 1.0, np.float32)
_sB = np.r_[np.full(64, 1.0, np.float32),
            np.full(64, 2.0, np.float32)]


def _prep_weights(inputs):
    f32 = np.float32
    W = {}
    for l in range(3):
        Wih = inputs[f'W_ih{l}'].astype(f32)
        Whh = inputs[f'W_hh{l}'].astype(f32)
        b = (inputs[f'b_ih{l}'] + inputs[f'b_hh{l}']).astype(f32)
        for perm, s, tag in ((_permA, _sA, 'A'), (_permB, _sB, 'B')):
            if l == 0:
                W[f'wx{tag}'] = (Wih[perm].T * s[None, :]).astype(BF16)
                m = np.zeros((128, 128), f32)
                m[64:128, :] = Whh[perm].T * s[None, :]
                W[f'w0{tag}'] = m.astype(BF16)
            else:
                m = np.concatenate([Wih[perm].T, Whh[perm].T], axis=0)
                m = m * s[None, :]
                W[f'w{l}{tag}'] = m.astype(BF16)
        W.setdefault('biasA', []).append(b[_permA] * _sA)
        W.setdefault('biasB', []).append(b[_permB] * _sB)
    W['bvals'] = np.stack(W.pop('biasA') + W.pop('biasB')).astype(BF16)
    # per-chain indicator [6, 2*NB]: layer l -> cols 16l:16l+16 (A region)
    # and NB+16l:... (B region)
    ind = np.zeros((6, 2 * NB), np.float32)
    for l in range(3):
        ind[l, BG * l:BG * l + BG] = 1.0
        ind[3 + l, NB + BG * l:NB + BG * l + BG] = 1.0
    W['ind'] = ind.astype(BF16)
    W['wout'] = inputs['W_out'].astype(f32).T.astype(BF16)  # [64, 2]
    return W


def _step_meta():
    meta = []
    dve = 3  # memsets (V0, V1, D) per chain happen on chain A=0/B=1 counters
    for s in range(T + 4):
        ls = [l for l in (0, 1, 2) if 0 <= s - 2 * l < T]
        c0, c1 = min(ls) * BG, (max(ls) + 1) * BG
        f0, f1 = c0, min(c1, 2 * BG)
        has_ff = f0 < f1
        nops = 4 + (1 if has_ff else 0)
        meta.append(dict(ls=ls, c0=c0, c1=c1, f0=f0, f1=f1, has_ff=has_ff,
                         dve_before=dve, dve_Q=dve + 2, dve_C=dve + 3,
                         dve_h=dve + 4))
        dve += nops
    return meta, dve


def _build_program():
    import concourse.bacc as bacc
    from concourse import mybir

    AF = mybir.ActivationFunctionType
    ALU = mybir.AluOpType
    fp16 = mybir.dt.float16
    f32 = mybir.dt.float32

    nc = bacc.Bacc(None, target_bir_lowering=False, debug=False)
    xT_d = nc.dram_tensor("xT", [128, T * BC], fp16, kind="ExternalInput")
    wnames = ['wxA', 'wxB', 'w0A', 'w0B', 'w1A', 'w1B', 'w2A', 'w2B']
    wall_d = nc.dram_tensor("wall", [128, 8 * 128 + 2], fp16, kind="ExternalInput")
    fall_d = nc.dram_tensor("fall", [6, 128 + 2 * NB], fp16, kind="ExternalInput")
    out_d = nc.dram_tensor("out", [2, BC], f32, kind="ExternalOutput")

    meta, dve_total = _step_meta()
    NCHUNK = T // XCHUNK

    from contextlib import ExitStack
    with ExitStack() as stack:
        e = stack.enter_context
        wall = e(nc.sbuf_tensor("wall_s", [128, 8 * 128 + 2], fp16))
        fall = e(nc.sbuf_tensor("fall_s", [6, 128 + 2 * NB], fp16))
        XB = e(nc.sbuf_tensor("XB", [128, NXBUF * XCHUNK * BC], fp16))
        H2f = e(nc.sbuf_tensor("H2f", [64, BC], fp16))
        outT = e(nc.sbuf_tensor("outT", [2, BC], f32))
        PO = e(nc.psum_tensor([2, BC], f32))
        dma_w = e(nc.semaphore("dma_w"))
        dma_x0 = e(nc.semaphore("dma_x0"))
        dma_x1 = e(nc.semaphore("dma_x1"))
        dma_x2 = e(nc.semaphore("dma_x2"))
        dma_x = [dma_x0, dma_x1, dma_x2]
        s_xc = e(nc.semaphore("s_xc"))
        s_ep = e(nc.semaphore("s_ep"))
        ch = []
        for g in range(G):
            V0 = e(nc.sbuf_tensor(f"V0_{g}", [128, NB], fp16))
            V1 = e(nc.sbuf_tensor(f"V1_{g}", [128, NB], fp16))
            Dt = e(nc.sbuf_tensor(f"D_{g}", [64, NB], fp16))
            Pt = e(nc.sbuf_tensor(f"Pt_{g}", [64, NB], fp16))
            Qt = e(nc.sbuf_tensor(f"Qt_{g}", [64, NB], fp16))
            TT0 = e(nc.sbuf_tensor(f"TT0_{g}", [128, 2 * NB], fp16))
            TT1 = e(nc.sbuf_tensor(f"TT1_{g}", [128, 2 * NB], fp16))
            TC = e(nc.sbuf_tensor(f"TC_{g}", [64, NB], fp16))
            PA0 = e(nc.psum_tensor([128, 2 * NB], f32))
            PA1 = e(nc.psum_tensor([128, 2 * NB], f32))
            s_pe = e(nc.semaphore(f"s_pe{g}"))
            s_act = e(nc.semaphore(f"s_act{g}"))
            s_dve = e(nc.semaphore(f"s_dve{g}"))
            d = dict(V0=V0, V1=V1, D=Dt, Pt=Pt, Qt=Qt, TT0=TT0, TT1=TT1,
                     TC=TC, PA0=PA0, PA1=PA1, s_pe=s_pe, s_act=s_act,
                     s_dve=s_dve)
            d['Vb'] = [V0, V1]
            d['TTb'] = [TT0, TT1]
            d['PAb'] = [PA0, PA1]
            ch.append(d)
        block = e(nc.Block())

        ws = {n: wall[:, 128 * k:128 * (k + 1)] for k, n in enumerate(wnames)}
        wout = wall[0:64, 8 * 128:8 * 128 + 2]
        bvals = fall[:, 0:128]
        ind = fall[:, 128:128 + 2 * NB]
        wA = {0: ws['w0A'], 1: ws['w1A'], 2: ws['w2A']}
        wB = {0: ws['w0B'], 1: ws['w1B'], 2: ws['w2B']}

        # ---- nominal schedule: (time, emit_closure) per engine ------------
        sched = {'pe': [], 'act': [], 'dve': []}

        def at(engine, t, fn):
            sched[engine].append((t, len(sched[engine]), fn))

        def t_of(g, s, off):
            return PERIOD * s + OFFSET * g + off

        for g in range(G):
            c = ch[g]
            for s in range(T + 4):
                m = meta[s]
                at('pe', t_of(g, s - 2, 1684), _mk_bias(nc, c, meta, s, bvals, ind, dma_w))
                if 0 in m['ls']:
                    at('pe', t_of(g, s - 2, 1724),
                       _mk_xmm(nc, c, s, g, ws, XB, dma_x, s_xc))
                at('pe', t_of(g, s - 1, 1604), _mk_rec(nc, c, meta, s, wA, wB))
                at('act', t_of(g, s, 0), _mk_sigma(nc, c, s, AF))
                at('act', t_of(g, s, 957), _mk_tanh(nc, c, meta, s, AF))
                at('dve', t_of(g, s, 483), _mk_X(nc, c, s, ALU))
                at('dve', t_of(g, s, 593), _mk_Q(nc, c, s))
                at('dve', t_of(g, s, 774), _mk_C(nc, c, meta, s))
                at('dve', t_of(g, s, 1400), _mk_h(nc, c, meta, s))
                if m['has_ff']:
                    at('dve', t_of(g, s, 1485), _mk_ff(nc, c, meta, s))
            # epilogue copy for this chain
            at('dve', t_of(g, T + 4, 100), _mk_epcopy(nc, c, g, H2f, dve_total, s_ep))
        # final matmul + copy out
        at('pe', t_of(1, T + 5, 0), _mk_epmm(nc, PO, wout, H2f, s_ep, ch))
        at('act', t_of(1, T + 5, 500), _mk_epout(nc, outT, PO, ch))

        for k in sched:
            sched[k].sort(key=lambda x: (x[0], x[1]))

        @block.sync
        def _(sync):
            sync.dma_start(out=wall[:, :], in_=wall_d[:, :]).then_inc(dma_w, 16)
            sync.dma_start(out=fall[:, :], in_=fall_d[:, :]).then_inc(dma_w, 16)
            for cc in range(NCHUNK):
                buf = cc % NXBUF
                ins = sync.dma_start(
                    out=XB[:, buf * XCHUNK * BC:(buf + 1) * XCHUNK * BC],
                    in_=xT_d[:, cc * XCHUNK * BC:(cc + 1) * XCHUNK * BC])
                if cc >= NXBUF:
                    # both chains must have consumed chunk cc-NXBUF
                    ins.wait_op(s_xc, G * (cc - NXBUF + 1), "sem-ge")
                ins.then_inc(dma_x[cc % NXBUF], 16)
            sync.dma_start(out=out_d[:, :], in_=outT[:, :]).wait_op(
                ch[0]['s_act'], 2 * (T + 4) + 1, "sem-ge").then_inc(dma_w, 16)

        @block.tensor
        def _(tensor):
            tensor.wait_ge(dma_w, 32)
            for t, k, fn in sched['pe']:
                fn()

        @block.scalar
        def _(scalar):
            for t, k, fn in sched['act']:
                fn()

        @block.vector
        def _(vector):
            for g in range(G):
                c = ch[g]
                nc.vector.memset(c['V0'][:, :], 0.0).then_inc(c['s_dve'], 1)
                nc.vector.memset(c['V1'][:, :], 0.0).then_inc(c['s_dve'], 1)
                nc.vector.memset(c['D'][:, :], 0.0).then_inc(c['s_dve'], 1)
            for t, k, fn in sched['dve']:
                fn()

    nc.compile()
    return nc


# ---- emit closures (default-arg capture) ----------------------------------

def _mk_bias(nc, c, meta, s, bvals, ind, dma_w):
    def f():
        i = nc.tensor.matmul(c['PAb'][s % 2][:, :], bvals, ind,
                             start=True, stop=False, skip_group_check=True)
        if s >= 2:
            i.wait_op(c['s_act'], 2 * (s - 2) + 1, "sem-ge")
    return f


def _mk_xmm(nc, c, s, g, ws, XB, dma_x, s_xc):
    def f():
        cc = s // XCHUNK
        buf = cc % NXBUF
        k = (s % XCHUNK) * BC + g * BG
        xs = XB[:, buf * XCHUNK * BC + k:buf * XCHUNK * BC + k + BG]
        pA = c['PAb'][s % 2]
        i = nc.tensor.matmul(pA[:, 0:BG], ws['wxA'], xs,
                             start=False, stop=False, skip_group_check=True)
        if s % XCHUNK == 0:
            i.wait_op(dma_x[cc % NXBUF], 16 * (cc // NXBUF + 1), "sem-ge")
        i2 = nc.tensor.matmul(pA[:, NB:NB + BG], ws['wxB'], xs,
                              start=False, stop=False, skip_group_check=True)
        if s % XCHUNK == XCHUNK - 1 or s == T - 1:
            i2.then_inc(s_xc, 1)
    return f


def _mk_rec(nc, c, meta, s, wA, wB):
    def f():
        m = meta[s]
        pA = c['PAb'][s % 2]
        V = c['Vb'][s % 2]
        first = True
        last = None
        for l in m['ls']:
            cl = slice(BG * l, BG * l + BG)
            clB = slice(NB + BG * l, NB + BG * l + BG)
            i = nc.tensor.matmul(pA[:, cl], wA[l], V[:, cl],
                                 start=False, stop=True, skip_group_check=True)
            if first:
                i.wait_op(c['s_dve'], meta[s - 1]['dve_h'] if s else 3, "sem-ge")
                first = False
            last = nc.tensor.matmul(pA[:, clB], wB[l], V[:, cl],
                                    start=False, stop=True, skip_group_check=True)
        last.then_inc(c['s_pe'], 1)
    return f


def _mk_sigma(nc, c, s, AF):
    def f():
        nc.scalar.activation(c['TTb'][s % 2][:, :], c['PAb'][s % 2][:, :],
                             AF.Sigmoid) \
            .wait_op(c['s_pe'], s + 1, "sem-ge").then_inc(c['s_act'], 1)
    return f


def _mk_tanh(nc, c, meta, s, AF):
    def f():
        nc.scalar.activation(c['TC'][:, :], c['D'][:, :], AF.Tanh, scale=2.0) \
            .wait_op(c['s_dve'], meta[s]['dve_C'], "sem-ge").then_inc(c['s_act'], 1)
    return f


def _mk_X(nc, c, s, ALU):
    def f():
        TT = c['TTb'][s % 2]
        nc.vector.scalar_tensor_tensor(
            c['Pt'][:, :], TT[64:128, NB:2 * NB], 0.5, TT[64:128, 0:NB],
            ALU.subtract, ALU.mult) \
            .wait_op(c['s_act'], 2 * s + 1, "sem-ge").then_inc(c['s_dve'], 1)
    return f


def _mk_Q(nc, c, s):
    def f():
        TT = c['TTb'][s % 2]
        nc.vector.tensor_mul(
            c['Qt'][:, :], TT[0:64, 0:NB], c['D'][:, :]).then_inc(c['s_dve'], 1)
    return f


def _mk_C(nc, c, meta, s):
    def f():
        m = meta[s]
        cs = slice(m['c0'], m['c1'])
        nc.vector.tensor_add(
            c['D'][:, cs], c['Pt'][:, cs], c['Qt'][:, cs]) \
            .wait_op(c['s_dve'], m['dve_Q'], "sem-ge").then_inc(c['s_dve'], 1)
    return f


def _mk_h(nc, c, meta, s):
    def f():
        m = meta[s]
        cs = slice(m['c0'], m['c1'])
        TT = c['TTb'][s % 2]
        nc.vector.tensor_mul(
            c['Vb'][(s + 1) % 2][64:128, cs],
            TT[0:64, NB + m['c0']:NB + m['c1']], c['TC'][:, cs]) \
            .wait_op(c['s_act'], 2 * s + 2, "sem-ge").then_inc(c['s_dve'], 1)
    return f


def _mk_ff(nc, c, meta, s):
    def f():
        m = meta[s]
        f0, f1 = m['f0'], m['f1']
        TT = c['TTb'][s % 2]
        nc.vector.tensor_mul(
            c['Vb'][s % 2][0:64, BG + f0:BG + f1],
            TT[0:64, NB + f0:NB + f1], c['TC'][:, f0:f1]).then_inc(c['s_dve'], 1)
    return f


def _mk_epcopy(nc, c, g, H2f, dve_total, s_ep):
    def f():
        nc.vector.tensor_copy(
            H2f[:, g * BG:(g + 1) * BG],
            c['Vb'][(T + 4) % 2][64:128, 2 * BG:3 * BG]) \
            .wait_op(c['s_dve'], dve_total, "sem-ge").then_inc(s_ep, 1)
    return f


def _mk_epmm(nc, PO, wout, H2f, s_ep, ch):
    def f():
        i = nc.tensor.matmul(PO[:, :], wout, H2f[:, :], start=True, stop=True)
        i.wait_op(s_ep, G, "sem-ge")
        i.then_inc(ch[0]['s_pe'], 1)
    return f


def _mk_epout(nc, outT, PO, ch):
    def f():
        nc.scalar.copy(outT[:, :], PO[:, :]) \
            .wait_op(ch[0]['s_pe'], T + 5, "sem-ge").then_inc(ch[0]['s_act'], 1)
    return f


def pack_operands(W):
    wall = np.zeros((128, 8 * 128 + 2), BF16)
    for k, n in enumerate(['wxA', 'wxB', 'w0A', 'w0B', 'w1A', 'w1B', 'w2A', 'w2B']):
        wall[:, 128 * k:128 * (k + 1)] = W[n]
    wall[0:64, 1024:1026] = W['wout']
    fall = np.zeros((6, 128 + 2 * NB), BF16)
    fall[:, 0:128] = W['bvals']
    fall[:, 128:128 + 2 * NB] = W['ind']
    return wall, fall


def make_in_maps(inputs):
    W = _prep_weights(inputs)
    wall, fall = pack_operands(W)
    x = inputs['x'].astype(np.float32)
    in_maps = []
    for c in range(NCORES):
        xc = x[c * BC:(c + 1) * BC]
        xT = np.ascontiguousarray(xc.transpose(2, 1, 0).reshape(I, T * BC)).astype(BF16)
        in_maps.append({'xT': xT, 'wall': wall, 'fall': fall})
    return in_maps


def kernel(**inputs):
    from concourse.bass_utils import run_bass_kernel_spmd

    inputs = {k: np.asarray(v) for k, v in inputs.items()}
    if 'nc' not in _cache:
        _cache['nc'] = _build_program()
    nc = _cache['nc']

    in_maps = make_in_maps(inputs)
    res = run_bass_kernel_spmd(nc, in_maps, list(range(NCORES)))
    outs = [res.results[c]['out'].T for c in range(NCORES)]
    full = np.concatenate(outs, axis=0).astype(np.float32)
    full = full + inputs['b_out'].astype(np.float32)[None, :]
    return full


# revision 16
# speedup vs baseline: 1.0266x; 1.0266x over previous
"""3-layer LSTM (B=256, T=512, I=128, H=64) + final linear, on 8 TRN2 NeuronCores.

G=2 pipelined variant: each core's 32-batch is split into two independent
16-batch chains (A, B) offset by OFFSET ns.  Each chain runs the skew-2
3-layer wavefront with its own gate tile [128, 96], cell state [64, 48],
PSUM bank pair, and semaphores; the two chains share the engines, weights
and x-chunk DMAs.  While chain A sits in a cross-engine latency gap
(write-commit tails + semaphore props dominate the serial recurrence),
chain B's work executes on the idle engines.

Per-engine instruction emission follows the nominal steady-state schedule
(period PERIOD, chain B at +OFFSET) because engine queues are FIFO with
head-of-line blocking: emission order must equal execution order.

Sync design (per chain): every instruction carries at most ONE attached
sem wait; WAR/WAW covered transitively by engine order; same-engine RAW
(X,Q -> C) gets an explicit self-wait because DVE writes commit only
after the pipeline drain.
"""
import numpy as np
import ml_dtypes

B, T, I, H = 256, 512, 128, 64
NCORES = 8
BC = B // NCORES            # 32 batch per core
G = 3
BGS = [11, 11, 10]          # uneven batch split per chain
BOFF = [0, 11, 22]
XCHUNK = 16
NXBUF = 3

PERIOD = 1767.0
OFFSETS = [0.0, 660.0, 1180.0]

BF16 = np.float16
_cache = {}

_permA = np.r_[64:128, 0:64]       # [f; i]
_permB = np.r_[192:256, 128:192]   # [o; g]
_sA = np.full(128, 1.0, np.float32)
_sB = np.r_[np.full(64, 1.0, np.float32),
            np.full(64, 2.0, np.float32)]


def _prep_weights(inputs):
    f32 = np.float32
    W = {}
    for l in range(3):
        Wih = inputs[f'W_ih{l}'].astype(f32)
        Whh = inputs[f'W_hh{l}'].astype(f32)
        b = (inputs[f'b_ih{l}'] + inputs[f'b_hh{l}']).astype(f32)
        for perm, s, tag in ((_permA, _sA, 'A'), (_permB, _sB, 'B')):
            if l == 0:
                W[f'wx{tag}'] = (Wih[perm].T * s[None, :]).astype(BF16)
                m = np.zeros((128, 128), f32)
                m[64:128, :] = Whh[perm].T * s[None, :]
                W[f'w0{tag}'] = m.astype(BF16)
            else:
                m = np.concatenate([Wih[perm].T, Whh[perm].T], axis=0)
                m = m * s[None, :]
                W[f'w{l}{tag}'] = m.astype(BF16)
        W.setdefault('biasA', []).append(b[_permA] * _sA)
        W.setdefault('biasB', []).append(b[_permB] * _sB)
    W['bvals'] = np.stack(W.pop('biasA') + W.pop('biasB')).astype(BF16)
    # per-chain indicators: one per distinct BG (11 and 10)
    for bg in (11, 10):
        nb = 3 * bg
        ind = np.zeros((6, 2 * nb), np.float32)
        for l in range(3):
            ind[l, bg * l:bg * l + bg] = 1.0
            ind[3 + l, nb + bg * l:nb + bg * l + bg] = 1.0
        W[f'ind{bg}'] = ind.astype(BF16)
    W['wout'] = inputs['W_out'].astype(f32).T.astype(BF16)  # [64, 2]
    return W


def _step_meta(bg):
    meta = []
    dve = 3  # memsets (V0, V1, D) precede the loop on each chain's counter
    for s in range(T + 4):
        ls = [l for l in (0, 1, 2) if 0 <= s - 2 * l < T]
        c0, c1 = min(ls) * bg, (max(ls) + 1) * bg
        f0, f1 = c0, min(c1, 2 * bg)
        has_ff = f0 < f1
        nops = 4 + (1 if has_ff else 0)
        meta.append(dict(ls=ls, c0=c0, c1=c1, f0=f0, f1=f1, has_ff=has_ff,
                         dve_before=dve, dve_Q=dve + 2, dve_C=dve + 3,
                         dve_h=dve + 4))
        dve += nops
    return meta, dve


def _build_program():
    import concourse.bacc as bacc
    from concourse import mybir

    AF = mybir.ActivationFunctionType
    ALU = mybir.AluOpType
    fp16 = mybir.dt.float16
    f32 = mybir.dt.float32

    nc = bacc.Bacc(None, target_bir_lowering=False, debug=False)
    xT_d = nc.dram_tensor("xT", [128, T * BC], fp16, kind="ExternalInput")
    wnames = ['wxA', 'wxB', 'w0A', 'w0B', 'w1A', 'w1B', 'w2A', 'w2B']
    wall_d = nc.dram_tensor("wall", [128, 8 * 128 + 2], fp16, kind="ExternalInput")
    fall_d = nc.dram_tensor("fall", [6, 128 + 66 + 60], fp16, kind="ExternalInput")
    out_d = nc.dram_tensor("out", [2, BC], f32, kind="ExternalOutput")

    metas = [_step_meta(bg) for bg in BGS]
    dve_total = metas[0][1]
    NCHUNK = T // XCHUNK

    from contextlib import ExitStack
    with ExitStack() as stack:
        e = stack.enter_context
        wall = e(nc.sbuf_tensor("wall_s", [128, 8 * 128 + 2], fp16))
        fall = e(nc.sbuf_tensor("fall_s", [6, 128 + 66 + 60], fp16))
        XB = e(nc.sbuf_tensor("XB", [128, NXBUF * XCHUNK * BC], fp16))
        H2f = e(nc.sbuf_tensor("H2f", [64, BC], fp16))
        outT = e(nc.sbuf_tensor("outT", [2, BC], f32))
        PO = e(nc.psum_tensor([2, BC], f32))
        dma_w = e(nc.semaphore("dma_w"))
        dma_x0 = e(nc.semaphore("dma_x0"))
        dma_x1 = e(nc.semaphore("dma_x1"))
        dma_x2 = e(nc.semaphore("dma_x2"))
        dma_x = [dma_x0, dma_x1, dma_x2]
        s_xc = e(nc.semaphore("s_xc"))
        s_ep = e(nc.semaphore("s_ep"))
        ch = []
        for g in range(G):
            bg = BGS[g]
            nb = 3 * bg
            V0 = e(nc.sbuf_tensor(f"V0_{g}", [128, nb], fp16))
            V1 = e(nc.sbuf_tensor(f"V1_{g}", [128, nb], fp16))
            Dt = e(nc.sbuf_tensor(f"D_{g}", [64, nb], fp16))
            Pt = e(nc.sbuf_tensor(f"Pt_{g}", [64, nb], fp16))
            Qt = e(nc.sbuf_tensor(f"Qt_{g}", [64, nb], fp16))
            TT0 = e(nc.sbuf_tensor(f"TT0_{g}", [128, 2 * nb], fp16))
            TT1 = e(nc.sbuf_tensor(f"TT1_{g}", [128, 2 * nb], fp16))
            TC = e(nc.sbuf_tensor(f"TC_{g}", [64, nb], fp16))
            PA0 = e(nc.psum_tensor([128, 2 * nb], f32))
            PA1 = e(nc.psum_tensor([128, 2 * nb], f32))
            s_pe = e(nc.semaphore(f"s_pe{g}"))
            s_act = e(nc.semaphore(f"s_act{g}"))
            s_dve = e(nc.semaphore(f"s_dve{g}"))
            d = dict(V0=V0, V1=V1, D=Dt, Pt=Pt, Qt=Qt, TT0=TT0, TT1=TT1,
                     TC=TC, PA0=PA0, PA1=PA1, s_pe=s_pe, s_act=s_act,
                     s_dve=s_dve, bg=bg, nb=nb, boff=BOFF[g],
                     meta=metas[g][0])
            d['Vb'] = [V0, V1]
            d['TTb'] = [TT0, TT1]
            d['PAb'] = [PA0, PA1]
            ch.append(d)
        block = e(nc.Block())

        ws = {n: wall[:, 128 * k:128 * (k + 1)] for k, n in enumerate(wnames)}
        wout = wall[0:64, 8 * 128:8 * 128 + 2]
        bvals = fall[:, 0:128]
        inds = {11: fall[:, 128:128 + 66], 10: fall[:, 128 + 66:128 + 126]}
        wA = {0: ws['w0A'], 1: ws['w1A'], 2: ws['w2A']}
        wB = {0: ws['w0B'], 1: ws['w1B'], 2: ws['w2B']}

        # ---- nominal schedule: (time, emit_closure) per engine ------------
        sched = {'pe': [], 'act': [], 'dve': []}

        def at(engine, t, fn):
            sched[engine].append((t, len(sched[engine]), fn))

        def t_of(g, s, off):
            return PERIOD * s + OFFSETS[g] + off

        for g in range(G):
            c = ch[g]
            meta = c['meta']
            nb, bg = c['nb'], c['bg']
            # per-chain nominal op times from the cost model
            Esig = 2 * nb * 0.8333 + 185
            EX = nb * 1.0417 + 60
            EQ = nb * 0.5208 + 60
            Etanh = nb * 0.8333 + 185
            Erec = 6 * bg * 0.4167
            tX = Esig + 218
            tQ = tX + EX
            tC = tQ + EQ + 96
            ttanh = tC + EQ + 98
            th = ttanh + Etanh + 218
            tff = th + EQ
            trec = th + EQ + 119
            for s in range(T + 4):
                m = meta[s]
                at('pe', t_of(g, s - 2, trec + Erec + 40), _mk_bias(nc, c, meta, s, bvals, inds[bg], dma_w))
                if 0 in m['ls']:
                    at('pe', t_of(g, s - 2, trec + Erec + 80),
                       _mk_xmm(nc, c, s, g, ws, XB, dma_x, s_xc))
                at('pe', t_of(g, s - 1, trec), _mk_rec(nc, c, meta, s, wA, wB))
                at('act', t_of(g, s, 0), _mk_sigma(nc, c, s, AF))
                at('act', t_of(g, s, ttanh), _mk_tanh(nc, c, meta, s, AF))
                at('dve', t_of(g, s, tX), _mk_X(nc, c, s, ALU))
                at('dve', t_of(g, s, tQ), _mk_Q(nc, c, s))
                at('dve', t_of(g, s, tC), _mk_C(nc, c, meta, s))
                at('dve', t_of(g, s, th), _mk_h(nc, c, meta, s))
                if m['has_ff']:
                    at('dve', t_of(g, s, tff), _mk_ff(nc, c, meta, s))
            # epilogue copy for this chain
            at('dve', t_of(g, T + 4, 100), _mk_epcopy(nc, c, g, H2f, dve_total, s_ep))
        # final matmul + copy out
        at('pe', t_of(1, T + 5, 0), _mk_epmm(nc, PO, wout, H2f, s_ep, ch))
        at('act', t_of(1, T + 5, 500), _mk_epout(nc, outT, PO, ch))

        for k in sched:
            sched[k].sort(key=lambda x: (x[0], x[1]))

        @block.sync
        def _(sync):
            sync.dma_start(out=wall[:, :], in_=wall_d[:, :]).then_inc(dma_w, 16)
            sync.dma_start(out=fall[:, :], in_=fall_d[:, :]).then_inc(dma_w, 16)
            for cc in range(NCHUNK):
                buf = cc % NXBUF
                ins = sync.dma_start(
                    out=XB[:, buf * XCHUNK * BC:(buf + 1) * XCHUNK * BC],
                    in_=xT_d[:, cc * XCHUNK * BC:(cc + 1) * XCHUNK * BC])
                if cc >= NXBUF:
                    # both chains must have consumed chunk cc-NXBUF
                    ins.wait_op(s_xc, G * (cc - NXBUF + 1), "sem-ge")
                ins.then_inc(dma_x[cc % NXBUF], 16)
            sync.dma_start(out=out_d[:, :], in_=outT[:, :]).wait_op(
                ch[0]['s_act'], 2 * (T + 4) + 1, "sem-ge").then_inc(dma_w, 16)

        @block.tensor
        def _(tensor):
            tensor.wait_ge(dma_w, 32)
            for t, k, fn in sched['pe']:
                fn()

        @block.scalar
        def _(scalar):
            for t, k, fn in sched['act']:
                fn()

        @block.vector
        def _(vector):
            for g in range(G):
                c = ch[g]
                nc.vector.memset(c['V0'][:, :], 0.0).then_inc(c['s_dve'], 1)
                nc.vector.memset(c['V1'][:, :], 0.0).then_inc(c['s_dve'], 1)
                nc.vector.memset(c['D'][:, :], 0.0).then_inc(c['s_dve'], 1)
            for t, k, fn in sched['dve']:
                fn()

    nc.compile()
    return nc


# ---- emit closures (default-arg capture) ----------------------------------

def _mk_bias(nc, c, meta, s, bvals, ind, dma_w):
    def f():
        i = nc.tensor.matmul(c['PAb'][s % 2][:, :], bvals, ind,
                             start=True, stop=False, skip_group_check=True)
        if s >= 2:
            i.wait_op(c['s_act'], 2 * (s - 2) + 1, "sem-ge")
    return f


def _mk_xmm(nc, c, s, g, ws, XB, dma_x, s_xc):
    def f():
        bg, nb = c['bg'], c['nb']
        cc = s // XCHUNK
        buf = cc % NXBUF
        k = (s % XCHUNK) * BC + c['boff']
        xs = XB[:, buf * XCHUNK * BC + k:buf * XCHUNK * BC + k + bg]
        pA = c['PAb'][s % 2]
        i = nc.tensor.matmul(pA[:, 0:bg], ws['wxA'], xs,
                             start=False, stop=False, skip_group_check=True)
        if s % XCHUNK == 0:
            i.wait_op(dma_x[cc % NXBUF], 16 * (cc // NXBUF + 1), "sem-ge")
        i2 = nc.tensor.matmul(pA[:, nb:nb + bg], ws['wxB'], xs,
                              start=False, stop=False, skip_group_check=True)
        if s % XCHUNK == XCHUNK - 1:
            i2.then_inc(s_xc, 1)
    return f


def _mk_rec(nc, c, meta, s, wA, wB):
    def f():
        bg, nb = c['bg'], c['nb']
        m = meta[s]
        pA = c['PAb'][s % 2]
        V = c['Vb'][s % 2]
        first = True
        last = None
        for l in m['ls']:
            cl = slice(bg * l, bg * l + bg)
            clB = slice(nb + bg * l, nb + bg * l + bg)
            i = nc.tensor.matmul(pA[:, cl], wA[l], V[:, cl],
                                 start=False, stop=True, skip_group_check=True)
            if first:
                i.wait_op(c['s_dve'], meta[s - 1]['dve_h'] if s else 3, "sem-ge")
                first = False
            last = nc.tensor.matmul(pA[:, clB], wB[l], V[:, cl],
                                    start=False, stop=True, skip_group_check=True)
        last.then_inc(c['s_pe'], 1)
    return f


def _mk_sigma(nc, c, s, AF):
    def f():
        nc.scalar.activation(c['TTb'][s % 2][:, :], c['PAb'][s % 2][:, :],
                             AF.Sigmoid) \
            .wait_op(c['s_pe'], s + 1, "sem-ge").then_inc(c['s_act'], 1)
    return f


def _mk_tanh(nc, c, meta, s, AF):
    def f():
        nc.scalar.activation(c['TC'][:, :], c['D'][:, :], AF.Tanh, scale=2.0) \
            .wait_op(c['s_dve'], meta[s]['dve_C'], "sem-ge").then_inc(c['s_act'], 1)
    return f


def _mk_X(nc, c, s, ALU):
    def f():
        nb = c['nb']
        TT = c['TTb'][s % 2]
        nc.vector.scalar_tensor_tensor(
            c['Pt'][:, :], TT[64:128, nb:2 * nb], 0.5, TT[64:128, 0:nb],
            ALU.subtract, ALU.mult) \
            .wait_op(c['s_act'], 2 * s + 1, "sem-ge").then_inc(c['s_dve'], 1)
    return f


def _mk_Q(nc, c, s):
    def f():
        nb = c['nb']
        TT = c['TTb'][s % 2]
        nc.vector.tensor_mul(
            c['Qt'][:, :], TT[0:64, 0:nb], c['D'][:, :]).then_inc(c['s_dve'], 1)
    return f


def _mk_C(nc, c, meta, s):
    def f():
        m = meta[s]
        cs = slice(m['c0'], m['c1'])
        nc.vector.tensor_add(
            c['D'][:, cs], c['Pt'][:, cs], c['Qt'][:, cs]) \
            .wait_op(c['s_dve'], m['dve_Q'], "sem-ge").then_inc(c['s_dve'], 1)
    return f


def _mk_h(nc, c, meta, s):
    def f():
        m = meta[s]
        cs = slice(m['c0'], m['c1'])
        nb = c['nb']
        TT = c['TTb'][s % 2]
        nc.vector.tensor_mul(
            c['Vb'][(s + 1) % 2][64:128, cs],
            TT[0:64, nb + m['c0']:nb + m['c1']], c['TC'][:, cs]) \
            .wait_op(c['s_act'], 2 * s + 2, "sem-ge").then_inc(c['s_dve'], 1)
    return f


def _mk_ff(nc, c, meta, s):
    def f():
        m = meta[s]
        f0, f1 = m['f0'], m['f1']
        bg, nb = c['bg'], c['nb']
        TT = c['TTb'][s % 2]
        nc.vector.tensor_mul(
            c['Vb'][s % 2][0:64, bg + f0:bg + f1],
            TT[0:64, nb + f0:nb + f1], c['TC'][:, f0:f1]).then_inc(c['s_dve'], 1)
    return f


def _mk_epcopy(nc, c, g, H2f, dve_total, s_ep):
    def f():
        bg = c['bg']
        nc.vector.tensor_copy(
            H2f[:, c['boff']:c['boff'] + bg],
            c['Vb'][(T + 4) % 2][64:128, 2 * bg:3 * bg]) \
            .wait_op(c['s_dve'], dve_total, "sem-ge").then_inc(s_ep, 1)
    return f


def _mk_epmm(nc, PO, wout, H2f, s_ep, ch):
    def f():
        i = nc.tensor.matmul(PO[:, :], wout, H2f[:, :], start=True, stop=True)
        i.wait_op(s_ep, G, "sem-ge")
        i.then_inc(ch[0]['s_pe'], 1)
    return f


def _mk_epout(nc, outT, PO, ch):
    def f():
        nc.scalar.copy(outT[:, :], PO[:, :]) \
            .wait_op(ch[0]['s_pe'], T + 5, "sem-ge").then_inc(ch[0]['s_act'], 1)
    return f


def pack_operands(W):
    wall = np.zeros((128, 8 * 128 + 2), BF16)
    for k, n in enumerate(['wxA', 'wxB', 'w0A', 'w0B', 'w1A', 'w1B', 'w2A', 'w2B']):
        wall[:, 128 * k:128 * (k + 1)] = W[n]
    wall[0:64, 1024:1026] = W['wout']
    fall = np.zeros((6, 128 + 66 + 60), BF16)
    fall[:, 0:128] = W['bvals']
    fall[:, 128:128 + 66] = W['ind11']
    fall[:, 128 + 66:128 + 126] = W['ind10']
    return wall, fall


def make_in_maps(inputs):
    W = _prep_weights(inputs)
    wall, fall = pack_operands(W)
    x = inputs['x'].astype(np.float32)
    in_maps = []
    for c in range(NCORES):
        xc = x[c * BC:(c + 1) * BC]
        xT = np.ascontiguousarray(xc.transpose(2, 1, 0).reshape(I, T * BC)).astype(BF16)
        in_maps.append({'xT': xT, 'wall': wall, 'fall': fall})
    return in_maps


def kernel(**inputs):
    from concourse.bass_utils import run_bass_kernel_spmd

    inputs = {k: np.asarray(v) for k, v in inputs.items()}
    if 'nc' not in _cache:
        _cache['nc'] = _build_program()
    nc = _cache['nc']

    in_maps = make_in_maps(inputs)
    res = run_bass_kernel_spmd(nc, in_maps, list(range(NCORES)))
    outs = [res.results[c]['out'].T for c in range(NCORES)]
    full = np.concatenate(outs, axis=0).astype(np.float32)
    full = full + inputs['b_out'].astype(np.float32)[None, :]
    return full
